# revision 1
# baseline (speedup 1.0000x reference)
"""Multi-head causal attention (dense transformer block) on 8 Trainium2 cores.

Sharding: 2-way data parallel over batch x 4-way tensor parallel over heads.
Core c handles batch c//4 and heads 4*(c%4) .. 4*(c%4)+3.

Per-core pipeline (all activation layouts chosen so no on-device transposes
are needed; host pre-transposes x and the weight shards once):
  1. QT/KT [hd, t] and V [t, hd] projections from xT [d, t]. Weight tiles
     arrive pre-packed on the host so one DMA carries two d-subtiles
     (halves the HWDGE descriptor load).
  2. Attention per (head, q-chunk) with scores computed transposed
     (S^T [k, q]), exp without max-subtraction (scores are O(1) so exp is
     safe in fp32), causal masking via affine_select on the diagonal tiles,
     softmax denominators via ones-vector matmuls, AV accumulated as
     out^T [hd, q].
  3. Output projection final[t, e] = sum_c out^T[c, t] * woT[c, e] (partial
     sum over this core's heads).
  4. ReduceScatter over the 4 cores sharing a batch; host concatenates the
     row shards.

All matmul inputs are float32r (full-rate fp32 PE mode, ~1.5e-4 relative
error per contraction). PSUM accumulation stays fp32. DMA issue is split
across the SP and ACT sequencers; PSUM evictions run on DVE.
"""

import os
import sys

sys.path.insert(0, "/opt/trn_rl_repo")

import numpy as np

N_CORES = 8
B = 2
T = 2048          # sequence length
D = 2048          # model dim
P = 128           # partitions
HD = 128          # head dim
NHG = 4           # head-groups (cores per batch)
HPC = 4           # heads per core
F = HPC * HD      # 512 per-core q/k/v feature width
TC = 512          # token chunk (matmul free dim)
NTC = T // TC     # 4 token chunks
ND = D // P       # 16 d-subtiles
NJ = ND // 2      # 8 packed weight tiles (2 d-subtiles each)
SCALE = float(HD) ** -0.5

_CACHE = {}


def _build(mm_dtype_name: str, reps: int = 1, with_rs: bool = True):
    import concourse.bacc as bacc
    import concourse.mybir as mybir
    import concourse.tile as tile

    dt = mybir.dt
    f32 = dt.float32
    md = getattr(dt, mm_dtype_name)  # dtype of every PE-input tile

    nc = bacc.Bacc(
        "TRN2", target_bir_lowering=False, debug=False, num_devices=N_CORES
    )

    xT = nc.dram_tensor("xT", [D, T], md, kind="ExternalInput")
    # packed: [j*128+p, sub*512+f] = W^T[(2j+sub)*128+p, f]
    wqP = nc.dram_tensor("wqP", [D // 2, 2 * F], md, kind="ExternalInput")
    wkP = nc.dram_tensor("wkP", [D // 2, 2 * F], md, kind="ExternalInput")
    wvP = nc.dram_tensor("wvP", [D // 2, 2 * F], md, kind="ExternalInput")
    woT = nc.dram_tensor("woT", [F, D], md, kind="ExternalInput")
    out = nc.dram_tensor("out", [T // NHG, D], f32, kind="ExternalOutput")

    with nc.allow_low_precision(reason="float32r matmul-input tiles"), \
         tile.TileContext(nc) as tc:
        with (
            tc.tile_pool(name="const", bufs=1) as const,
            tc.tile_pool(name="resident", bufs=1) as res_pool,
            tc.tile_pool(name="dram", bufs=1, space="DRAM") as dram,
        ):
            ones_stage = const.tile([P, P], f32)
            nc.vector.memset(ones_stage[:], 1.0)
            ones_col = const.tile([P, 1], md)
            nc.scalar.copy(ones_col[:], ones_stage[:, 0:1])
            ones_row = const.tile([1, P], md)
            nc.scalar.copy(ones_row[:], ones_stage[0:1, :])

            # ---- resident activation buffers ----
            QT = [res_pool.tile([P, T], md, name=f"QT{h}") for h in range(HPC)]
            KT = [res_pool.tile([P, T], md, name=f"KT{h}") for h in range(HPC)]
            V = [res_pool.tile([P, F], md, name=f"V{i}") for i in range(T // P)]

            bounce = [dram.tile([TC, D], f32, name=f"bounce{qt}")
                      for qt in range(NTC - 1)]
            bounce += [dram.tile([TC // 2, D], f32, name=f"bounce3{hf}")
                       for hf in range(2)]
            rs_out = [dram.tile([TC // NHG, D], f32, name=f"rs_out{qt}")
                      for qt in range(NTC - 1)]
            rs_out += [dram.tile([TC // 2 // NHG, D], f32, name=f"rs_out3{hf}")
                       for hf in range(2)]

            for rep in range(reps):
                _build_body(nc, tc, mybir, md, f32, rep,
                            xT, wqP, wkP, wvP, woT, out,
                            ones_col, ones_row, QT, KT, V,
                            bounce, rs_out, with_rs)

    nc.compile()
    return nc


def _build_body(nc, tc, mybir, md, f32, rep,
                xT, wqP, wkP, wvP, woT, out,
                ones_col, ones_row, QT, KT, V,
                bounce, rs_out, with_rs=True):
    # ---- phase 1: projections ----
    # Two supersteps of 1024 tokens; each loads the packed q/k/v weights
    # once (24 MB instead of 48 MB of weight traffic per pass over x).
    TG = 2 * TC
    with tc.tile_pool(name=f"psum1_{rep}", bufs=1, space="PSUM") as psum1, \
         tc.tile_pool(name=f"xw_{rep}", bufs=3) as xw_pool:
        for tg in range(T // TG):
            xts = []
            for di in range(ND):
                xt = xw_pool.tile(
                    [P, TG], md, name=f"xt_{rep}_{tg}_{di}", tag="xt",
                    bufs=ND + 2,
                )
                nc.sync.dma_start(
                    xt[:],
                    xT.ap()[di * P:(di + 1) * P, tg * TG:(tg + 1) * TG],
                )
                xts.append(xt)
            wts = {}
            for wname, wP in (("q", wqP), ("k", wkP), ("v", wvP)):
                for j in range(NJ):
                    wt = xw_pool.tile(
                        [P, 2 * F], md, name=f"w{wname}_{rep}_{tg}_{j}",
                        tag="wt", bufs=6,
                    )
                    nc.scalar.dma_start(wt[:], wP.ap()[j * P:(j + 1) * P, :])
                    wts[wname, j] = wt
            for wname, dest in (("q", QT), ("k", KT)):
                pss = [
                    psum1.tile(
                        [P, TC], f32, name=f"ps_{wname}{h}{th}_{rep}_{tg}",
                        tag="pq", bufs=8,
                    )
                    for h in range(HPC) for th in range(2)
                ]
                for j in range(NJ):
                    wt = wts[wname, j]
                    for sub in range(2):
                        di = 2 * j + sub
                        for h in range(HPC):
                            for th in range(2):
                                nc.tensor.matmul(
                                    pss[2 * h + th][:],
                                    wt[:, sub * F + h * HD:
                                       sub * F + (h + 1) * HD],
                                    xts[di][:, th * TC:(th + 1) * TC],
                                    start=(di == 0),
                                    stop=(di == ND - 1),
                                )
                for h in range(HPC):
                    for th in range(2):
                        col = tg * TG + th * TC
                        nc.any.tensor_copy(
                            dest[h][:, col:col + TC], pss[2 * h + th][:]
                        )
            pss = [
                psum1.tile(
                    [P, F], f32, name=f"ps_v{ts}_{rep}_{tg}", tag="pq", bufs=8
                )
                for ts in range(TG // P)
            ]
            for j in range(NJ):
                wt = wts["v", j]
                for sub in range(2):
                    di = 2 * j + sub
                    for ts in range(TG // P):
                        nc.tensor.matmul(
                            pss[ts][:],
                            xts[di][:, ts * P:(ts + 1) * P],
                            wt[:, sub * F:(sub + 1) * F],
                            start=(di == 0),
                            stop=(di == ND - 1),
                        )
            for ts in range(TG // P):
                nc.any.tensor_copy(V[tg * (TG // P) + ts][:], pss[ts][:])

    # ---- phases 2+3 per q chunk ----
    with tc.tile_pool(name=f"psum2_{rep}", bufs=1, space="PSUM") as psum2, \
         tc.tile_pool(name=f"work_{rep}", bufs=6) as work:
        WO = []
        for ci in range(HPC):
            row = []
            for etp in range(NTC // 2):
                wo = work.tile([P, 2 * TC], md, name=f"WO{rep}_{ci}_{etp}",
                               tag=f"WO{ci}_{etp}", bufs=1)
                nc.sync.dma_start(
                    wo[:],
                    woT.ap()[ci * P:(ci + 1) * P,
                             etp * 2 * TC:(etp + 1) * 2 * TC],
                )
                row.append(wo)
            WO.append(row)
        for qt in range(NTC):
            outT = {}
            n_k = (qt + 1) * (TC // P)  # causal: k-subtiles needed
            diag0 = qt * (TC // P)
            korder = list(range(diag0, n_k)) + list(range(diag0))
            SKEW = 2
            for hp in (0, 2):  # head pairs, emission interleaved
                heads = (hp, hp + 1)
                ps_out = {
                    h: psum2.tile(
                        [P, TC], f32, name=f"ps_out{rep}_{qt}_{h}",
                        tag="out", bufs=2,
                    )
                    for h in heads
                }
                ps_den = {
                    h: psum2.tile(
                        [1, TC], f32, name=f"ps_den{rep}_{qt}_{h}",
                        tag="aux", bufs=2,
                    )
                    for h in heads
                }
                pts = {}
                # For diagonal tiles only columns q >= 128*dj are live:
                # S/exp/AV/den all operate on that sub-rectangle (the dead
                # region is never read, so it needs no zeroing), and the
                # causal mask shrinks to one 128x128 triangle block. korder
                # starts at dj=0 (full width), so the start=True matmuls
                # initialize every psum column's has_written bit.
                def live0(kt):
                    # clamp at TC-256: float32r matmuls need >=256 moving
                    # columns for full rate, so narrower is never faster
                    dj = kt - diag0
                    return min(max(0, dj) * P, TC - 2 * P)
                for step in range(n_k + SKEW):
                    if step < n_k:
                        kt = korder[step]
                        c0 = live0(kt)
                        for h in heads:
                            ps_st = psum2.tile(
                                [P, TC], f32,
                                name=f"ps_st{rep}_{qt}_{h}_{kt}",
                                tag="st", bufs=2,
                            )
                            nc.tensor.matmul(
                                ps_st[:, c0:],
                                KT[h][:, kt * P:(kt + 1) * P],
                                QT[h][:, qt * TC + c0:(qt + 1) * TC],
                                start=True,
                                stop=True,
                            )
                            pt = work.tile(
                                [P, TC], md, name=f"pt{rep}_{qt}_{h}_{kt}",
                                tag="pt", bufs=8,
                            )
                            nc.scalar.activation(
                                pt[:, c0:], ps_st[:, c0:],
                                mybir.ActivationFunctionType.Exp,
                                scale=SCALE,
                            )
                            dj = kt - diag0
                            if dj >= 0:
                                # mask [c0, (dj+1)*128): the dead strip below
                                # the triangle plus the triangle block itself
                                me = (dj + 1) * P
                                nc.gpsimd.affine_select(
                                    pt[:, c0:me], pt[:, c0:me],
                                    pattern=[[1, me - c0]],
                                    compare_op=mybir.AluOpType.is_ge,
                                    fill=0.0,
                                    base=-(dj * P - c0),
                                    channel_multiplier=-1,
                                )
                            pts[h, kt] = pt
                    if step >= SKEW:
                        idx = step - SKEW
                        k = korder[idx]
                        c0 = live0(k)
                        for h in heads:
                            nc.tensor.matmul(
                                ps_den[h][:, c0:],
                                ones_col[:],
                                pts[h, k][:, c0:],
                                start=(idx == 0),
                                stop=(idx == n_k - 1),
                            )
                            nc.tensor.matmul(
                                ps_out[h][:, c0:],
                                V[k][:, h * HD:(h + 1) * HD],
                                pts[h, k][:, c0:],
                                start=(idx == 0),
                                stop=(idx == n_k - 1),
                            )
                for h in heads:
                    den = work.tile([1, TC], md, name=f"den{rep}_{qt}_{h}",
                                    tag="den", bufs=2)
                    nc.vector.reciprocal(den[:], ps_den[h][:])
                    ps_bc = psum2.tile(
                        [P, TC], f32, name=f"ps_bc{rep}_{qt}_{h}", tag="aux",
                        bufs=2,
                    )
                    nc.tensor.matmul(
                        ps_bc[:], ones_row[:], den[:],
                        start=True, stop=True,
                    )
                    bc = work.tile([P, TC], f32, name=f"bc{rep}_{qt}_{h}",
                                   tag="bc", bufs=2)
                    nc.any.tensor_copy(bc[:], ps_bc[:])
                    ot = work.tile([P, TC], md, name=f"outT{rep}_{qt}_{h}",
                                   tag="outT", bufs=4)
                    nc.vector.tensor_mul(ot[:], ps_out[h][:], bc[:])
                    outT[h] = ot

            # output projection for this q(=t) chunk (resident weights).
            # ts-outer so bounce rows complete incrementally; the last
            # chunk's reduce-scatter runs in two half-sized pieces so its
            # exposed tail is halved.
            last = qt == NTC - 1
            for ts in range(TC // P):
                for etp in range(NTC // 2):
                    fin = work.tile(
                        [P, 2 * TC], f32, name=f"fin{rep}_{qt}_{ts}_{etp}",
                        tag="fin", bufs=3,
                    )
                    psf = [
                        psum2.tile(
                            [P, TC], f32,
                            name=f"ps_f{rep}_{qt}_{ts}_{etp}_{ee}",
                            tag="f", bufs=2,
                        )
                        for ee in range(2)
                    ]
                    for ci in range(HPC):
                        for ee in range(2):
                            nc.tensor.matmul(
                                psf[ee][:],
                                outT[ci][:, ts * P:(ts + 1) * P],
                                WO[ci][etp][:, ee * TC:(ee + 1) * TC],
                                start=(ci == 0),
                                stop=(ci == HPC - 1),
                            )
                    for ee in range(2):
                        nc.any.tensor_copy(
                            fin[:, ee * TC:(ee + 1) * TC], psf[ee][:]
                        )
                    if last:
                        dst = bounce[NTC - 1 + ts // 2]
                        drow = (ts % 2) * P
                    else:
                        dst = bounce[qt]
                        drow = ts * P
                    nc.sync.dma_start(
                        dst[drow:drow + P,
                            etp * 2 * TC:(etp + 1) * 2 * TC],
                        fin[:],
                    )
                if last and ts % 2 == 1 and with_rs:
                    hf = ts // 2
                    nc.gpsimd.collective_compute(
                        "ReduceScatter",
                        mybir.AluOpType.add,
                        replica_groups=[[0, 1, 2, 3], [4, 5, 6, 7]],
                        ins=[bounce[NTC - 1 + hf].opt()],
                        outs=[rs_out[NTC - 1 + hf].opt()],
                    )
                    rw = TC // 2 // NHG
                    base = qt * (TC // NHG) + hf * rw
                    nc.sync.dma_start(
                        out.ap()[base:base + rw, :],
                        rs_out[NTC - 1 + hf][:],
                    )
            # ---- phase 4: chunked reduce-scatter, overlapped with the
            # next chunk's compute. Core r of each batch group ends up with
            # rows qt*512 + r*128 .. +128; the host interleaves accordingly.
            if not last:
                if with_rs:
                    nc.gpsimd.collective_compute(
                        "ReduceScatter",
                        mybir.AluOpType.add,
                        replica_groups=[[0, 1, 2, 3], [4, 5, 6, 7]],
                        ins=[bounce[qt].opt()],
                        outs=[rs_out[qt].opt()],
                    )
                    nc.sync.dma_start(
                        out.ap()[qt * (TC // NHG):(qt + 1) * (TC // NHG), :],
                        rs_out[qt][:],
                    )
                else:
                    nc.sync.dma_start(
                        out.ap()[qt * (TC // NHG):(qt + 1) * (TC // NHG), :],
                        bounce[qt][0:TC // NHG, :],
                    )
            elif not with_rs:
                for hf in range(2):
                    rw = TC // 2 // NHG
                    base = qt * (TC // NHG) + hf * rw
                    nc.sync.dma_start(
                        out.ap()[base:base + rw, :],
                        bounce[NTC - 1 + hf][0:rw, :],
                    )




def _get_nc():
    name = os.environ.get("ATTN_MM_DTYPE", "float32r")
    reps = int(os.environ.get("ATTN_REPS", "1"))
    key = (name, reps)
    if key not in _CACHE:
        _CACHE[key] = _build(name, reps)
    return _CACHE[key]


last_exec_time_ns = None


def _pack_w(wT):
    # [2048, 512] -> [1024, 1024]: packed[j*128+p, sub*512+f] =
    # wT[(2j+sub)*128+p, f]
    return np.ascontiguousarray(
        wT.reshape(NJ, 2, P, F).swapaxes(1, 2).reshape(D // 2, 2 * F)
    )


def make_in_maps(x, w_qkv, w_out):
    x = np.asarray(x, dtype=np.float32)
    w_qkv = np.asarray(w_qkv, dtype=np.float32)
    w_out = np.asarray(w_out, dtype=np.float32)
    xTs = [np.ascontiguousarray(x[b].T) for b in range(B)]
    in_maps = []
    for c in range(N_CORES):
        b, hg = divmod(c, NHG)
        sl = slice(hg * F, (hg + 1) * F)
        in_maps.append({
            "xT": xTs[b],
            "wqP": _pack_w(w_qkv[0 * D:1 * D][sl].T),
            "wkP": _pack_w(w_qkv[1 * D:2 * D][sl].T),
            "wvP": _pack_w(w_qkv[2 * D:3 * D][sl].T),
            "woT": np.ascontiguousarray(w_out[:, sl].T),
        })
    return in_maps


def kernel(x, w_qkv, w_out):
    import time

    from concourse import bass_utils

    global last_exec_time_ns
    nc = _get_nc()
    in_maps = make_in_maps(x, w_qkv, w_out)

    trace = bool(int(os.environ.get("ATTN_TRACE", "0")))
    res = None
    last_err = None
    for attempt in range(3):
        try:
            res = bass_utils.run_bass_kernel_spmd(
                nc, in_maps, core_ids=list(range(N_CORES)), trace=trace
            )
            break
        except Exception as e:  # transient axon mesh desyncs
            last_err = e
            time.sleep(10 * (attempt + 1))
    if res is None:
        raise last_err
    last_exec_time_ns = res.exec_time_ns

    outs = [res.results[c]["out"] for c in range(N_CORES)]
    # chunked RS layout: core r of a batch group holds, for chunks 0..2,
    # the summed rows qt*TC + r*128 .. +128; for the split last chunk it
    # holds rows 3*TC + hf*256 + r*64 .. +64 for hf in {0, 1}.
    RW = TC // NHG
    full = []
    for b in range(B):
        arr = np.stack(outs[b * NHG:(b + 1) * NHG])      # [r, NTC*RW, D]
        fb = np.empty((T, D), np.float32)
        head = arr[:, :(NTC - 1) * RW].reshape(NHG, NTC - 1, RW, D)
        fb[:(NTC - 1) * TC] = head.transpose(1, 0, 2, 3).reshape(-1, D)
        tail = arr[:, (NTC - 1) * RW:].reshape(NHG, 2, RW // 2, D)
        fb[(NTC - 1) * TC:] = tail.transpose(1, 0, 2, 3).reshape(-1, D)
        full.append(fb)
    return np.stack(full)

